# revision 14
# baseline (speedup 1.0000x reference)
"""MoE routing (BaseLayer) Trainium2 kernel.

Problem: B=1, S=1024, D=1024, F=4096, E=8 experts, 8 NeuronCores.
  feats[T,D] -> scores=feats@centroids.T -> expert=argmax -> alpha=sigmoid(max score)
  y = per-token assigned expert's BaseSublayer(feats):  xs + relu(LN(xs)*g+b @ W1.T + b1) @ W2.T + b2
  out = alpha*y + (1-alpha)*feats

Sharding: expert-parallel — core e owns expert e's weights. Routing (the tiny
[T,D]x[D,E] scores matmul + argmax + gather) runs on host inside kernel();
each core receives only ITS expert's tokens, transposed [D, cap].  The device
kernel runs entirely in the transposed layout so no on-chip transposes are
needed:
  - LN stats: ones-vector matmuls reduce over D (partition dim) -> [1,cap] rows
  - per-token scale/shift rows broadcast to 128 partitions via K=1 matmuls
  - ff1: out[F,cap] = W1T.T @ xn   (weights stationary, tokens moving, bf16)
  - ff2: out[D,cap] = W2T.T @ relu1
  - residual+gate fused on DVE; result DMA'd out still transposed; host scatters.
gamma/beta are folded into W1/b1 on host (exact algebra, not an approximation).
"""

import sys
import types

import numpy as np
import ml_dtypes


def _ensure_axon_hooks():
    """bass_utils imports antenv.axon_hooks when BASS_TRACE is set; some
    images lack that module. Provide a minimal registry so the import
    never crashes (hook stays None unless someone registers one)."""
    try:
        import antenv.axon_hooks  # noqa: F401
        return
    except ImportError:
        pass
    try:
        import antenv
    except ImportError:
        return
    mod = types.ModuleType("antenv.axon_hooks")
    mod._hook = None

    def set_axon_ntff_profile_hook(h):
        mod._hook = h

    def get_axon_ntff_profile_hook():
        return mod._hook

    mod.set_axon_ntff_profile_hook = set_axon_ntff_profile_hook
    mod.get_axon_ntff_profile_hook = get_axon_ntff_profile_hook
    sys.modules["antenv.axon_hooks"] = mod
    antenv.axon_hooks = mod


_ensure_axon_hooks()

import concourse.bass as bass  # noqa: E402
import concourse.bacc as bacc  # noqa: E402
import concourse.mybir as mybir  # noqa: E402
import concourse.tile as tile  # noqa: E402
from concourse.bass_utils import run_bass_kernel_spmd  # noqa: E402

F32 = mybir.dt.float32
BF16 = mybir.dt.bfloat16
EPS = 1e-5

D = 1024
F = 4096
E = 8
KD = D // 128   # 8 k-tiles over D
KF = F // 128   # 32 k-tiles over F

_CACHE = {}
LAST_RESULTS = None


def build_nc(cap: int):
    """Build the SPMD Bass program for per-core token capacity `cap`.

    PSUM budget (8 banks): ff1 uses tags ps1_0..3, ff2 uses ps2_0..3;
    the early stats/broadcast tiles alias those tags so at most 8 banks
    are ever needed.
    """
    assert cap % 32 == 0 and 32 <= cap <= 512, f"cap={cap} unsupported"
    GM1 = 4                      # ff1 m-tiles per PSUM group (4 banks)
    GM2 = 4                      # ff2 m-tiles per PSUM group (4 banks)
    n_g1 = KF // GM1             # 8 groups
    n_g2 = KD // GM2             # 2 groups

    # Bacc (not raw Bass): its compile() runs move_matmul_waits_to_ldweights
    # + generate_event_semaphores — TRN2 allows only ONE sync wait per
    # instruction, and walrus rejects the raw Tile output otherwise.
    nc = bacc.Bacc("TRN2", target_bir_lowering=False, debug=False)
    xtT = nc.declare_dram_parameter("xtT", [D, cap], F32, isOutput=False)
    alr = nc.declare_dram_parameter("alpha", [1, cap], F32, isOutput=False)
    w1t = nc.declare_dram_parameter("w1t", [D, F], BF16, isOutput=False)
    w2t = nc.declare_dram_parameter("w2t", [F, D], BF16, isOutput=False)
    b1d = nc.declare_dram_parameter("b1", [128, KF], F32, isOutput=False)
    b2d = nc.declare_dram_parameter("b2", [128, KD], F32, isOutput=False)
    outd = nc.declare_dram_parameter("out", [D, cap], F32, isOutput=True)

    with tile.TileContext(nc) as tc:
        with (
            tc.tile_pool(name="wsb", bufs=1) as wpool,
            tc.tile_pool(name="xsb", bufs=1) as xpool,
            tc.tile_pool(name="work", bufs=2) as work,
            tc.tile_pool(name="ps_f1", bufs=1, space="PSUM") as ps_f1,
            tc.tile_pool(name="ps_f2", bufs=1, space="PSUM") as ps_f2,
        ):
            v = nc.vector
            A = mybir.AluOpType

            # ---- input DMAs ------------------------------------------------
            xts = []
            for k in range(KD):
                xt = xpool.tile([128, cap], F32, name=f"xt{k}", tag=f"xt{k}")
                nc.sync.dma_start(out=xt[:], in_=xtT[k * 128:(k + 1) * 128, :])
                xts.append(xt)
            al_row = xpool.tile([1, cap], F32, name="al_row", tag="al_row")
            nc.sync.dma_start(out=al_row[:], in_=alr[:, :])
            b1sb = xpool.tile([128, KF], F32, name="b1sb", tag="b1sb")
            nc.sync.dma_start(out=b1sb[:], in_=b1d[:, :])
            b2sb = xpool.tile([128, KD], F32, name="b2sb", tag="b2sb")
            nc.sync.dma_start(out=b2sb[:], in_=b2d[:, :])

            # W1T: [128,1024] column-pair chunks, pair-major so ff1 group g
            # only waits for column pair g//2 (2KB partition lines, 256KB
            # per DMA).  W2T: full [128,1024] rows (k-major matches ff2's
            # k-inner consumption order).
            w1sb = {}
            for gp in range(n_g1 // 2):
                for k in range(KD):
                    t = wpool.tile([128, 2 * GM1 * 128], BF16,
                                   name=f"w1_{k}_{gp}", tag=f"w1_{k}_{gp}")
                    nc.sync.dma_start(
                        out=t[:],
                        in_=w1t[k * 128:(k + 1) * 128,
                                gp * 1024:(gp + 1) * 1024])
                    w1sb[(k, gp)] = t
            w2sb = {}
            for k in range(KF):
                t = wpool.tile([128, D], BF16, name=f"w2_{k}", tag=f"w2_{k}")
                nc.sync.dma_start(out=t[:],
                                  in_=w2t[k * 128:(k + 1) * 128, :])
                w2sb[k] = t

            # ---- constants -------------------------------------------------
            ones_col = xpool.tile([128, 1], F32, name="ones_col", tag="ones_col")
            v.memset(ones_col[:], 1.0)
            ones_row = xpool.tile([1, 128], F32, name="ones_row", tag="ones_row")
            v.memset(ones_row[:], 1.0)
            eps1 = xpool.tile([1, 1], F32, name="eps1", tag="eps1")
            v.memset(eps1[:], EPS)

            # ---- LN stats: sum and sumsq over D via ones-matmuls -----------
            stat_s = ps_f2.tile([1, cap], F32, name="stat_s", tag="ps2_0")
            stat_q = ps_f2.tile([1, cap], F32, name="stat_q", tag="ps2_1")
            for k in range(KD):
                sq = work.tile([128, cap], F32, name="sq", tag="sq")
                v.tensor_tensor(sq[:], xts[k][:], xts[k][:], A.mult)
                nc.tensor.matmul(stat_s[:], ones_col[:], xts[k][:],
                                 start=(k == 0), stop=(k == KD - 1))
                nc.tensor.matmul(stat_q[:], ones_col[:], sq[:],
                                 start=(k == 0), stop=(k == KD - 1))

            mu = work.tile([1, cap], F32, name="mu", tag="mu")
            v.tensor_scalar_mul(mu[:], stat_s[:], 1.0 / D)
            msq = work.tile([1, cap], F32, name="msq", tag="msq")
            v.tensor_scalar_mul(msq[:], stat_q[:], 1.0 / D)
            var = work.tile([1, cap], F32, name="var", tag="var")
            # var = msq - mu^2  (fused: (mu * -1) * mu, then + msq)
            v.scalar_tensor_tensor(var[:], mu[:], -1.0, mu[:], A.mult, A.mult)
            v.tensor_tensor(var[:], var[:], msq[:], A.add)
            std = work.tile([1, cap], F32, name="std", tag="std")
            nc.scalar.activation(std[:], var[:],
                                 mybir.ActivationFunctionType.Sqrt,
                                 bias=eps1[:])
            rstd = work.tile([1, cap], F32, name="rstd", tag="rstd")
            v.reciprocal(rstd[:], std[:])
            nmr = work.tile([1, cap], F32, name="nmr", tag="nmr")
            # nmr = -mu * rstd
            v.scalar_tensor_tensor(nmr[:], mu[:], -1.0, rstd[:], A.mult, A.mult)

            # ---- broadcast rows to all 128 partitions (K=1 matmuls) --------
            bc_a_ps = ps_f1.tile([128, cap], F32, name="bc_a_ps", tag="ps1_0")
            nc.tensor.matmul(bc_a_ps[:], ones_row[:], rstd[:],
                             start=True, stop=True)
            bc_b_ps = ps_f1.tile([128, cap], F32, name="bc_b_ps", tag="ps1_1")
            nc.tensor.matmul(bc_b_ps[:], ones_row[:], nmr[:],
                             start=True, stop=True)
            bc_al_ps = ps_f1.tile([128, cap], F32, name="bc_al_ps", tag="ps1_2")
            nc.tensor.matmul(bc_al_ps[:], ones_row[:], al_row[:],
                             start=True, stop=True)
            a_bc = xpool.tile([128, cap], F32, name="a_bc", tag="a_bc")
            v.tensor_copy(a_bc[:], bc_a_ps[:])
            b_bc = xpool.tile([128, cap], F32, name="b_bc", tag="b_bc")
            v.tensor_copy(b_bc[:], bc_b_ps[:])
            al_bc = xpool.tile([128, cap], F32, name="al_bc", tag="al_bc")
            v.tensor_copy(al_bc[:], bc_al_ps[:])

            # ---- normalize: xn = x*rstd - mu*rstd  (bf16) ------------------
            xns = []
            for k in range(KD):
                t = work.tile([128, cap], F32, name="xnt", tag="xnt")
                v.tensor_tensor(t[:], xts[k][:], a_bc[:], A.mult)
                xn = xpool.tile([128, cap], BF16, name=f"xn{k}", tag=f"xn{k}")
                v.tensor_tensor(xn[:], t[:], b_bc[:], A.add)
                xns.append(xn)

            # ---- ff1: relu1[m] = relu(W1'[m] @ xn + b1[m]),  m in 0..31 ----
            relu1 = []
            for g in range(n_g1):
                gp, gh = g // 2, g % 2
                pst = [ps_f1.tile([128, cap], F32,
                                  name=f"ps1_{g}_{j}", tag=f"ps1_{j}")
                       for j in range(GM1)]
                for k in range(KD):
                    w = w1sb[(k, gp)]
                    for j in range(GM1):
                        c0 = gh * GM1 * 128 + j * 128
                        nc.tensor.matmul(pst[j][:], w[:, c0:c0 + 128],
                                         xns[k][:],
                                         start=(k == 0), stop=(k == KD - 1))
                for j in range(GM1):
                    m = g * GM1 + j
                    r = xpool.tile([128, cap], BF16, name=f"r{m}", tag=f"r{m}")
                    nc.scalar.activation(r[:], pst[j][:],
                                         mybir.ActivationFunctionType.Relu,
                                         bias=b1sb[:, m:m + 1])
                    relu1.append(r)

            # ---- ff2 + bias + gate + residual ------------------------------
            for g in range(n_g2):
                pst = [ps_f2.tile([128, cap], F32,
                                  name=f"ps2_{g}_{j}", tag=f"ps2_{j}")
                       for j in range(GM2)]
                for k in range(KF):
                    w = w2sb[k]
                    for j in range(GM2):
                        c0 = (g * GM2 + j) * 128
                        nc.tensor.matmul(pst[j][:], w[:, c0:c0 + 128],
                                         relu1[k][:],
                                         start=(k == 0), stop=(k == KF - 1))
                for j in range(GM2):
                    m = g * GM2 + j
                    # s2 = (ff2 + b2) * alpha ; out = s2 + x
                    s2 = work.tile([128, cap], F32, name="s2", tag="s2")
                    v.scalar_tensor_tensor(s2[:], pst[j][:], b2sb[:, m:m + 1],
                                           al_bc[:], A.add, A.mult)
                    o = work.tile([128, cap], F32, name="osb", tag="osb")
                    v.tensor_tensor(o[:], s2[:], xts[m][:], A.add)
                    nc.sync.dma_start(out=outd[m * 128:(m + 1) * 128, :],
                                      in_=o[:])
    nc.compile()
    return nc


def _get_nc(cap: int):
    if cap not in _CACHE:
        _CACHE[cap] = build_nc(cap)
    return _CACHE[cap]


def kernel(x, centroids, gamma, beta, W1, b1, W2, b2):
    B, S, Din = x.shape
    T = B * S
    feats = np.ascontiguousarray(x.reshape(T, Din), dtype=np.float32)

    # ---- host routing (0.1% of the FLOPs) ------------------------------
    scores = feats @ centroids.astype(np.float32).T          # [T, E]
    expert = scores.argmax(axis=1)
    top = scores[np.arange(T), expert].astype(np.float32)
    alpha = 1.0 / (1.0 + np.exp(-top))

    idxs = [np.nonzero(expert == e)[0] for e in range(E)]
    maxc = max(1, max(len(i) for i in idxs))
    cap = ((maxc + 31) // 32) * 32

    # ---- fold gamma/beta into W1/b1 (exact) ----------------------------
    W1f = W1.astype(np.float32) * gamma.astype(np.float32)[:, None, :]
    b1f = b1.astype(np.float32) + np.einsum(
        'efd,ed->ef', W1.astype(np.float32), beta.astype(np.float32))

    in_maps = []
    for e in range(E):
        idx = idxs[e]
        n = len(idx)
        xtT = np.zeros((Din, cap), np.float32)
        xtT[:, :n] = feats[idx].T
        al = np.zeros((1, cap), np.float32)
        al[0, :n] = alpha[idx]
        w1t_e = np.ascontiguousarray(W1f[e].T).astype(ml_dtypes.bfloat16)
        w2t_e = np.ascontiguousarray(W2[e].astype(np.float32).T).astype(
            ml_dtypes.bfloat16)
        b1_e = np.ascontiguousarray(b1f[e].reshape(KF, 128).T)
        b2_e = np.ascontiguousarray(b2[e].astype(np.float32).reshape(KD, 128).T)
        in_maps.append({
            "xtT": xtT, "alpha": al, "w1t": w1t_e, "w2t": w2t_e,
            "b1": b1_e, "b2": b2_e,
        })

    nc = _get_nc(cap)
    res = run_bass_kernel_spmd(nc, in_maps, list(range(E)))
    global LAST_RESULTS
    LAST_RESULTS = res

    out = np.empty((T, Din), np.float32)
    for e in range(E):
        idx = idxs[e]
        if len(idx):
            out[idx] = res.results[e]["out"][:, :len(idx)].T
    return out.reshape(x.shape).astype(x.dtype, copy=False)
